# revision 19
# baseline (speedup 1.0000x reference)
"""Trainium2 Bass kernel for nn_CustomLstm (D=2048, H=1024), 8-core tensor-parallel.

Sharding: all five weights/biases and outputs are sharded along the units (row)
dimension of W across 8 NeuronCores (256 rows each).  The (D,D) concat
activation is replicated; gate elementwise ops are local; ht is all-gathered
(in 4 column chunks, fp8) so the final w5 @ ht matmul + row softmax is local.

Precision: all five matmuls run as fp8e4 DoubleRow (operands pre-scaled by
32 host-side; the 1/1024 descale is folded into the activation's scale
operand, with biases pre-scaled by 1024).  The tanh candidate gate (w3) —
whose unit slope would double the bare-fp8 quantization error — is
compensated with first-order residuals: z3 = w3h@x8 + w3h@x8l + w3l@x8,
where x8l/w3l are the fp8 quantization residuals stored at the same scale,
so all three products accumulate in one PSUM group at DoubleRow rate.
Outputs are written fp16 and upcast on host.

DMA instruction count is minimized (the descriptor generator costs ~0.6us
per DMA regardless of size): the four phase-A biases are concatenated
host-side into one tensor loaded with one DMA per 128-row half, outputs are
accumulated full-width in SBUF and stored once per half, and the all-gather
read-back is one rearrange DMA per column chunk.
"""

import numpy as np
import ml_dtypes

import concourse.bass as bass
import concourse.bacc as bacc
import concourse.mybir as mybir
import concourse.tile as tile
import concourse.bass_utils as bass_utils

BF16 = ml_dtypes.bfloat16
F8 = ml_dtypes.float8_e4m3

D = 2048          # units == input dim of each weight matrix
N_CORES = 8
R = D // N_CORES  # 256 rows per core
PK = D // 128     # 16 contraction chunks of 128
NN = 4            # 4 column chunks of 512
NCOL = D // NN    # 512
NM = R // 128     # 2 row chunks of 128

S = 32.0          # fp8 operand scale; psum carries S*S = 1024x
SS = S * S
G3 = "res8"       # candidate gate: "f16" | "b16" | "res8" | "res8b" | "plain8"

_CACHE = None


def _build(reps=1, single=False, fake_ag=False, g3=None):
    g3 = g3 or G3
    nc = bacc.Bacc("TRN2", target_bir_lowering=False, debug=False,
                   num_devices=1 if single else N_CORES)
    f32 = mybir.dt.float32
    f16 = mybir.dt.float16
    bf16 = mybir.dt.bfloat16
    fp8 = mybir.dt.float8e4
    AF = mybir.ActivationFunctionType
    DR = mybir.MatmulPerfMode.DoubleRow

    concat8 = nc.dram_tensor("concat8", [D, D], fp8, kind="ExternalInput").ap()
    concat8l = (nc.dram_tensor("concat8l", [D, D], fp8,
                               kind="ExternalInput").ap()
                if g3.startswith("res8") else None)
    g3w = {"f16": f16, "b16": bf16}.get(g3)
    concat16 = (nc.dram_tensor("concat16", [D, D], g3w,
                               kind="ExternalInput").ap()
                if g3w else None)
    wt = [nc.dram_tensor(f"w{g}t", [D, R], g3w if (g == 3 and g3w)
                         else fp8, kind="ExternalInput").ap()
          for g in range(1, 6)]
    w3lt = (nc.dram_tensor("w3lt", [D, R], fp8, kind="ExternalInput").ap()
            if g3.startswith("res8") else None)
    # b1..b4 (all *1024) concatenated along columns, bf16
    ba = nc.dram_tensor("ba", [R, 4 * D], bf16, kind="ExternalInput").ap()
    b5 = nc.dram_tensor("b5", [R, D], bf16, kind="ExternalInput").ap()
    cprev = nc.dram_tensor("cprev", [R, D], bf16, kind="ExternalInput").ap()

    ct_o = nc.dram_tensor("ct_o", [R, D], f16, kind="ExternalOutput").ap()
    ht_o = nc.dram_tensor("ht_o", [R, D], f16, kind="ExternalOutput").ap()
    yt_o = nc.dram_tensor("yt_o", [R, D], f16, kind="ExternalOutput").ap()

    rg = [list(range(N_CORES))]

    def mm_pairs(ps, w_sb, x_sb, m, start, stop):
        """K=2048 fp8 DoubleRow pass of one (weights, x) product."""
        for kk in range(PK // 2):
            lhs = (w_sb[:, 2 * kk * R:(2 * kk + 2) * R]
                   .rearrange("p (two m) -> p two m", two=2)
                   [:, :, m * 128:(m + 1) * 128])
            rhs = (x_sb[:, 2 * kk * NCOL:(2 * kk + 2) * NCOL]
                   .rearrange("p (two n) -> p two n", two=2))
            nc.tensor.matmul(ps[:], lhs, rhs,
                             start=start and kk == 0,
                             stop=stop and kk == PK // 2 - 1, perf_mode=DR)

    def mm_f16(ps, w_sb, x_sb, m):
        for k in range(PK):
            nc.tensor.matmul(
                ps[:],
                w_sb[:, k * R + m * 128:k * R + (m + 1) * 128],
                x_sb[:, k * NCOL:(k + 1) * NCOL],
                start=(k == 0), stop=(k == PK - 1))

    with tile.TileContext(nc) as tc:
        with (
            tc.tile_pool(name="wpool", bufs=1) as wpool,
            tc.tile_pool(name="xpool", bufs=2) as xpool,
            tc.tile_pool(name="hpool", bufs=2) as hpool,
            tc.tile_pool(name="bpool", bufs=1) as bpool,
            tc.tile_pool(name="opool", bufs=1) as opool,
            tc.tile_pool(name="gpool", bufs=1) as gpool,
            tc.tile_pool(name="zpool", bufs=1) as zpool,
            tc.tile_pool(name="spool", bufs=4) as spool,
            tc.tile_pool(name="psum", bufs=1, space="PSUM") as pp,
            tc.tile_pool(name="dram", bufs=1, space="DRAM") as dram,
        ):
            for rep in range(reps):
                w_sb = [wpool.tile([128, PK * R],
                                   g3w if (g == 2 and g3w) else fp8,
                                   name=f"w{g}sb", tag=f"w{g}sb")
                        for g in range(5)]
                w3l_sb = (wpool.tile([128, PK * R], fp8, name="w3lsb",
                                     tag="w3lsb")
                          if g3.startswith("res8") else None)
                w_loaded = [False] * 5

                def load_w(g):
                    if not w_loaded[g]:
                        w_loaded[g] = True
                        if g == 0:
                            # 4-k pieces so the PE can start on k=0 early
                            for k4 in range(0, PK, 4):
                                nc.scalar.dma_start(
                                    w_sb[g][:, k4 * R:(k4 + 4) * R]
                                    .rearrange("p (k m) -> p k m", m=R),
                                    wt[g][k4 * 128:(k4 + 4) * 128, :]
                                    .rearrange("(k p) m -> p k m", p=128))
                        else:
                            nc.scalar.dma_start(
                                w_sb[g][:].rearrange("p (k m) -> p k m", m=R),
                                wt[g].rearrange("(k p) m -> p k m", p=128))
                            if g == 2 and g3.startswith("res8"):
                                nc.scalar.dma_start(
                                    w3l_sb[:].rearrange("p (k m) -> p k m",
                                                        m=R),
                                    w3lt.rearrange("(k p) m -> p k m", p=128))

                # weight prefetch in first-use order (all on the scalar queue,
                # which carries only dependency-free constant loads)
                for g in (0, 1, 3, 2):
                    load_w(g)

                # phase-A constants: one DMA per 128-row half each
                ba_sb = [bpool.tile([128, 4 * D], bf16, name=f"ba{m}",
                                    tag=f"ba{m}") for m in range(NM)]
                cp_sb = [bpool.tile([128, D], bf16, name=f"cp{m}",
                                    tag=f"cp{m}") for m in range(NM)]
                b5_sb = [bpool.tile([128, D], bf16, name=f"b5_{m}",
                                    tag=f"b5_{m}") for m in range(NM)]
                for m in range(NM):
                    rsl = slice(m * 128, (m + 1) * 128)
                    nc.scalar.dma_start(ba_sb[m][:], ba[rsl, :])
                    nc.scalar.dma_start(cp_sb[m][:], cprev[rsl, :])
                # full-width output accumulation tiles (one store per half)
                ct_f = [opool.tile([128, D], f16, name=f"ctf{m}", tag=f"ctf{m}")
                        for m in range(NM)]
                ht_f = [opool.tile([128, D], f16, name=f"htf{m}", tag=f"htf{m}")
                        for m in range(NM)]

                ag_outs = []
                # --- phase A: gates, ct, ht; all-gather ht by column chunk ---
                for n in range(NN):
                    csl = slice(n * NCOL, (n + 1) * NCOL)
                    x8_sb = xpool.tile([128, PK * NCOL], fp8, name=f"x8{n}",
                                       tag="x8sb")
                    if g3.startswith("res8"):
                        x2_sb = xpool.tile([128, PK * NCOL], fp8,
                                           name=f"x8l{n}", tag="x2sb")
                        x2_src = concat8l
                    elif g3w:
                        x2_sb = xpool.tile([128, PK * NCOL], g3w,
                                           name=f"x16{n}", tag="x2sb")
                        x2_src = concat16
                    else:
                        x2_sb = x2_src = None
                    if n == 0:
                        # 4-k pieces so gate0 starts early
                        for k4 in range(0, PK, 4):
                            nc.sync.dma_start(
                                x8_sb[:, k4 * NCOL:(k4 + 4) * NCOL]
                                .rearrange("p (k c) -> p k c", c=NCOL),
                                concat8[k4 * 128:(k4 + 4) * 128, csl]
                                .rearrange("(k p) c -> p k c", p=128))
                        for k8 in (range(0, PK, 8) if x2_sb is not None
                                   else ()):
                            nc.sync.dma_start(
                                x2_sb[:, k8 * NCOL:(k8 + 8) * NCOL]
                                .rearrange("p (k c) -> p k c", c=NCOL),
                                x2_src[k8 * 128:(k8 + 8) * 128, csl]
                                .rearrange("(k p) c -> p k c", p=128))
                    else:
                        nc.sync.dma_start(
                            x8_sb[:].rearrange("p (k c) -> p k c", c=NCOL),
                            concat8[:, csl].rearrange("(k p) c -> p k c", p=128))
                        if x2_sb is not None:
                            nc.sync.dma_start(
                                x2_sb[:].rearrange("p (k c) -> p k c", c=NCOL),
                                x2_src[:, csl].rearrange("(k p) c -> p k c",
                                                         p=128))

                    asp = "Local" if (single or fake_ag) else "Shared"
                    if n == NN - 1 and not (single or fake_ag):
                        h2 = NCOL // 2
                        ag_in = [dram.tile([R, h2], fp8, name=f"agin{n}{s}",
                                           tag=f"agin{n}{s}")
                                 for s in ("a", "b")]
                        ag_out = [dram.tile([D, h2], fp8, name=f"agout{n}{s}",
                                            tag=f"agout{n}{s}", addr_space=asp)
                                  for s in ("a", "b")]
                        ag_outs.append([(ag_out[0], 0, h2),
                                        (ag_out[1], h2, h2)])
                    else:
                        ag_in = dram.tile([R, NCOL], fp8, name=f"agin{n}",
                                          tag=f"agin{n}")
                        ag_out = dram.tile([D, NCOL], fp8, name=f"agout{n}",
                                           tag=f"agout{n}", addr_space=asp)
                        ag_outs.append([(ag_out, 0, NCOL)])

                    # fp8 gates first; the (slower) fp16 tanh gate last, with
                    # the independent elementwise work interleaved behind it
                    FNS = [AF.Sigmoid, AF.Sigmoid, AF.Tanh, AF.Sigmoid]
                    for m in range(NM):
                        rsl = slice(m * 128, (m + 1) * 128)
                        ps = [None] * 4
                        for g in (0, 1, 3, 2):
                            p = pp.tile([128, NCOL], f32, name=f"ps{g}",
                                        tag=f"ps{g}")
                            if g == 2 and g3 == "res8":
                                mm_pairs(p, w_sb[2], x8_sb, m, True, False)
                                mm_pairs(p, w_sb[2], x2_sb, m, False, False)
                                mm_pairs(p, w3l_sb, x8_sb, m, False, True)
                            elif g == 2 and g3 == "res8b":
                                mm_pairs(p, w_sb[2], x8_sb, m, True, True)
                                p2 = pp.tile([128, NCOL], f32, name="ps2b",
                                             tag="ps2b")
                                mm_pairs(p2, w_sb[2], x2_sb, m, True, False)
                                mm_pairs(p2, w3l_sb, x8_sb, m, False, True)
                                ps2b = p2
                            elif g == 2 and g3w:
                                mm_f16(p, w_sb[2], x2_sb, m)
                            else:
                                mm_pairs(p, w_sb[g], x8_sb, m, True, True)
                            ps[g] = p
                        load_w(4)  # w5 queued early; needed only in phase C

                        acts = [None] * 4

                        def bias_act(g):
                            psg = ps[g][:]
                            if g == 2 and g3 == "res8b":
                                t = gpool.tile([128, NCOL], f32, name="g3t",
                                               tag="g3t")
                                nc.vector.tensor_add(t[:], psg, ps2b[:])
                                psg = t[:]
                            pre = gpool.tile([128, NCOL], bf16, name=f"pre{g}",
                                             tag=f"pre{g}")
                            nc.vector.tensor_add(
                                pre[:], psg,
                                ba_sb[m][:, g * D + n * NCOL:
                                         g * D + (n + 1) * NCOL])
                            act = gpool.tile([128, NCOL], bf16, name=f"act{g}",
                                             tag=f"act{g}")
                            sc = 1.0 if (g == 2 and g3w) else 1.0 / SS
                            nc.scalar.activation(act[:], pre[:], FNS[g],
                                                 scale=sc)
                            acts[g] = act

                        for g in (0, 1, 3):
                            bias_act(g)
                        # t1 only needs gate0 — runs while the fp16 gate mms
                        t1 = gpool.tile([128, NCOL], bf16, name="t1", tag="t1")
                        nc.vector.tensor_mul(t1[:], acts[0][:],
                                             cp_sb[m][:, csl])
                        bias_act(2)
                        t2 = gpool.tile([128, NCOL], bf16, name="t2", tag="t2")
                        nc.vector.tensor_mul(t2[:], acts[1][:], acts[2][:])
                        nc.vector.tensor_add(ct_f[m][:, csl], t1[:], t2[:])

                        th = gpool.tile([128, NCOL], bf16, name="th", tag="th")
                        nc.scalar.activation(th[:], ct_f[m][:, csl], AF.Tanh)
                        nc.vector.tensor_mul(ht_f[m][:, csl], acts[3][:], th[:])
                        htb = gpool.tile([128, NCOL], fp8, name="htb",
                                         tag="htb", bufs=2)
                        nc.scalar.activation(htb[:], ht_f[m][:, csl], AF.Copy,
                                             scale=S)
                        if isinstance(ag_in, list):
                            h2 = NCOL // 2
                            nc.gpsimd.dma_start(ag_in[0][rsl, :], htb[:, :h2])
                            nc.gpsimd.dma_start(ag_in[1][rsl, :], htb[:, h2:])
                        else:
                            nc.gpsimd.dma_start(ag_in[rsl, :], htb[:])
                        if n == NN - 1:
                            nc.gpsimd.dma_start(ct_o[rsl, :], ct_f[m][:])
                            nc.gpsimd.dma_start(ht_o[rsl, :], ht_f[m][:])

                    if single or fake_ag:
                        # stand-in for the AllGather: equivalent local HBM
                        # write volume so TimelineSim sees the same DMA load
                        for blk in range(N_CORES):
                            eng = nc.sync if blk % 2 == 0 else nc.scalar
                            eng.dma_start(
                                ag_out[blk * R:(blk + 1) * R, :], ag_in[:])
                    elif isinstance(ag_in, list):
                        for agi, ago in zip(ag_in, ag_out):
                            nc.gpsimd.collective_compute(
                                "AllGather", mybir.AluOpType.bypass,
                                replica_groups=rg,
                                ins=[agi.opt()], outs=[ago.opt()])
                    else:
                        nc.gpsimd.collective_compute(
                            "AllGather", mybir.AluOpType.bypass,
                            replica_groups=rg,
                            ins=[ag_in.opt()], outs=[ag_out.opt()])

                # --- phase C: z5 = w5 @ ht + b5, then row softmax ---
                for m in range(NM):
                    nc.scalar.dma_start(b5_sb[m][:],
                                        b5[m * 128:(m + 1) * 128, :])
                exs = [zpool.tile([128, D], f32, name=f"ex{m}", tag=f"ex{m}")
                       for m in range(NM)]
                sms = [[spool.tile([128, 1], f32, name=f"sm{m}_{n}",
                                   tag=f"sm{m}_{n}") for n in range(NN)]
                       for m in range(NM)]
                for n in range(NN):
                    csl = slice(n * NCOL, (n + 1) * NCOL)
                    h_sb = hpool.tile([128, PK * NCOL], fp8, name=f"h{n}",
                                      tag="hsb")
                    for ago, coff, cw in ag_outs[n]:
                        nc.scalar.dma_start(
                            h_sb[:].rearrange("p (k c) -> p k c", c=NCOL)
                            [:, :, coff:coff + cw],
                            ago.rearrange("(k p) c -> p k c", p=128))
                    for m in range(NM):
                        p5 = pp.tile([128, NCOL], f32, name="ps5", tag="ps5",
                                     bufs=2)
                        mm_pairs(p5, w_sb[4], h_sb, m, True, True)
                        z5 = gpool.tile([128, NCOL], f32, name="z5", tag="z5",
                                        bufs=2)
                        nc.vector.tensor_add(z5[:], p5[:],
                                             b5_sb[m][:, csl])
                        # chunked exp (scale folds away the 1024x) with
                        # per-chunk row-sum; logits are bounded (|z| < ~1)
                        # so exp without max subtraction is safe
                        nc.scalar.activation(exs[m][:, csl], z5[:],
                                             AF.Exp, scale=1.0 / SS,
                                             accum_out=sms[m][n][:])

                yt_f = [opool.tile([128, D], f16, name=f"ytf{m}", tag=f"ytf{m}")
                        for m in range(NM)]
                for m in range(NM):
                    s01 = spool.tile([128, 1], f32, name="s01", tag="s01")
                    nc.vector.tensor_add(s01[:], sms[m][0][:], sms[m][1][:])
                    s23 = spool.tile([128, 1], f32, name="s23", tag="s23")
                    nc.vector.tensor_add(s23[:], sms[m][2][:], sms[m][3][:])
                    sm_t = spool.tile([128, 1], f32, name="sm_t", tag="sm_t")
                    nc.vector.tensor_add(sm_t[:], s01[:], s23[:])
                    rs = spool.tile([128, 1], f32, name="rs", tag="rs")
                    nc.vector.reciprocal(rs[:], sm_t[:])
                    for j in range(NN):
                        jsl = slice(j * NCOL, (j + 1) * NCOL)
                        nc.vector.tensor_scalar_mul(yt_f[m][:, jsl],
                                                    exs[m][:, jsl], rs[:])
                    nc.gpsimd.dma_start(yt_o[m * 128:(m + 1) * 128, :],
                                        yt_f[m][:])

    nc.compile()
    return nc


_RUNNER = None


def _build_runner(nc):
    """Cached jit-compiled SPMD executor mirroring run_bass_kernel_spmd's
    axon/PJRT path, so repeat kernel() calls skip retracing."""
    import jax
    from jax.sharding import Mesh, PartitionSpec, NamedSharding
    from jax.experimental.shard_map import shard_map
    from concourse.bass2jax import (_bass_exec_p, install_neuronx_cc_hook,
                                    partition_id_tensor)

    install_neuronx_cc_hook()
    partition_name = (nc.partition_id_tensor.name
                      if nc.partition_id_tensor else None)
    in_names, out_names, out_avals = [], [], []
    for alloc in nc.m.functions[0].allocations:
        if not isinstance(alloc, mybir.MemoryLocationSet):
            continue
        name = alloc.memorylocations[0].name
        if alloc.kind == "ExternalInput":
            if name != partition_name:
                in_names.append(name)
        elif alloc.kind == "ExternalOutput":
            out_names.append(name)
            out_avals.append(jax.core.ShapedArray(
                tuple(alloc.tensor_shape), mybir.dt.np(alloc.dtype)))
    n_params, n_outs = len(in_names), len(out_names)
    all_in = tuple(in_names + out_names
                   + ([partition_name] if partition_name else []))

    def _body(*args):
        operands = list(args)
        if partition_name is not None:
            operands.append(partition_id_tensor())
        return tuple(_bass_exec_p.bind(
            *operands, out_avals=tuple(out_avals), in_names=all_in,
            out_names=tuple(out_names), lowering_input_output_aliases=(),
            sim_require_finite=True, sim_require_nnan=True, nc=nc))

    devices = jax.devices()[:N_CORES]
    mesh = Mesh(np.asarray(devices), ("core",))
    specs = (PartitionSpec("core"),) * (n_params + n_outs)
    fn = jax.jit(
        shard_map(_body, mesh=mesh, in_specs=specs,
                  out_specs=(PartitionSpec("core"),) * n_outs,
                  check_rep=False),
        donate_argnums=tuple(range(n_params, n_params + n_outs)),
        keep_unused=True)
    sh = NamedSharding(mesh, PartitionSpec("core"))
    zeros = [np.zeros((N_CORES * av.shape[0], *av.shape[1:]), av.dtype)
             for av in out_avals]

    def run(in_maps):
        gin = [jax.device_put(
            np.concatenate([in_maps[c][nm] for c in range(N_CORES)], 0), sh)
            for nm in in_names]
        gz = [jax.device_put(z, sh) for z in zeros]
        out = fn(*gin, *gz)
        got = {nm: np.asarray(o) for nm, o in zip(out_names, out)}
        return [{nm: got[nm].reshape(N_CORES, -1, got[nm].shape[-1])[c]
                 for nm in out_names} for c in range(N_CORES)]

    return run


def _make_in_maps(inputs, g3=None):
    g3 = g3 or G3
    inp = {k: np.asarray(v) for k, v in inputs.items()}
    concat = np.concatenate([inp["hPrev"], inp["xt"]], axis=0).astype(np.float32)
    concat8 = (concat * S).astype(F8)
    in_maps = []
    for i in range(N_CORES):
        r = slice(i * R, (i + 1) * R)
        m = {"concat8": concat8,
             "cprev": np.ascontiguousarray(inp["cPrev"][r]).astype(BF16)}
        if g3.startswith("res8"):
            m["concat8l"] = (concat * S
                             - concat8.astype(np.float32)).astype(F8)
        elif g3 in ("f16", "b16"):
            m["concat16"] = concat.astype(
                np.float16 if g3 == "f16" else BF16)
        bs = []
        for g in range(1, 6):
            wT = np.ascontiguousarray(inp[f"w{g}"][r].astype(np.float32).T)
            bg = np.ascontiguousarray(inp[f"b{g}"][r]).astype(np.float32)
            ws = wT * S
            if g == 3 and g3 in ("f16", "b16"):
                m["w3t"] = wT.astype(
                    np.float16 if g3 == "f16" else BF16)
                bs.append(bg)          # unscaled: 16-bit gate psum is unscaled
                continue
            m[f"w{g}t"] = ws.astype(F8)
            if g == 3 and g3.startswith("res8"):
                m["w3lt"] = (ws - m["w3t"].astype(np.float32)).astype(F8)
            if g == 5:
                m["b5"] = (bg * SS).astype(BF16)
            else:
                bs.append(bg * SS)
        m["ba"] = np.concatenate(bs, axis=1).astype(BF16)
        in_maps.append(m)
    return in_maps


def kernel(**inputs):
    global _CACHE, _RUNNER
    if _CACHE is None:
        _CACHE = _build()
    nc = _CACHE
    in_maps = _make_in_maps(inputs)

    results = None
    if _RUNNER is not False:
        try:
            if _RUNNER is None:
                _RUNNER = _build_runner(nc)
            results = _RUNNER(in_maps)
        except Exception:
            _RUNNER = False  # fall back permanently for this process
    if results is None:
        res = bass_utils.run_bass_kernel_spmd(nc, in_maps,
                                              core_ids=list(range(N_CORES)))
        results = res.results

    ct = np.concatenate([results[i]["ct_o"] for i in range(N_CORES)], 0)
    ht = np.concatenate([results[i]["ht_o"] for i in range(N_CORES)], 0)
    yt = np.concatenate([results[i]["yt_o"] for i in range(N_CORES)], 0)
    return (ct.astype(np.float32), ht.astype(np.float32),
            yt.astype(np.float32))


# revision 22
# speedup vs baseline: 2.2042x; 2.2042x over previous
"""Trainium2 Bass kernel for nn_CustomLstm (D=2048, H=1024), 8-core tensor-parallel.

Sharding: all five weights/biases and outputs are sharded along the units (row)
dimension of W across 8 NeuronCores (256 rows each).  The (D,D) concat
activation is replicated; gate elementwise ops are local; ht is all-gathered
(in 4 column chunks, fp8) so the final w5 @ ht matmul + row softmax is local.

Precision: all five matmuls run as fp8e4 DoubleRow (operands pre-scaled by
32 host-side; the 1/1024 descale is folded into the activation's scale
operand, with biases pre-scaled by 1024).  The tanh candidate gate (w3) —
whose unit slope would double the bare-fp8 quantization error — is
compensated with first-order residuals: z3 = w3h@x8 + w3h@x8l + w3l@x8,
where x8l/w3l are the fp8 quantization residuals stored at the same scale,
so all three products accumulate in one PSUM group at DoubleRow rate.
Outputs are written fp16 and upcast on host.

DMA instruction count is minimized (the descriptor generator costs ~0.6us
per DMA regardless of size): the four phase-A biases are concatenated
host-side into one tensor loaded with one DMA per 128-row half, outputs are
accumulated full-width in SBUF and stored once per half, and the all-gather
read-back is one rearrange DMA per column chunk.
"""

import numpy as np
import ml_dtypes

import concourse.bass as bass
import concourse.bacc as bacc
import concourse.mybir as mybir
import concourse.tile as tile
import concourse.bass_utils as bass_utils

BF16 = ml_dtypes.bfloat16
F8 = ml_dtypes.float8_e4m3

D = 2048          # units == input dim of each weight matrix
N_CORES = 8
R = D // N_CORES  # 256 rows per core
PK = D // 128     # 16 contraction chunks of 128
NN = 4            # 4 column chunks of 512
NCOL = D // NN    # 512
NM = R // 128     # 2 row chunks of 128

S = 32.0          # fp8 operand scale; psum carries S*S = 1024x
SS = S * S
G3 = "b16"        # candidate gate: "f16" | "b16" | "res8" | "res8b" | "plain8"
AGM = "chunk"     # all-gather granularity: "chunk" (5 collectives) | "one"
PROBE = ""        # perf probes: "dve2" / "act2" duplicate that engine's work

_CACHE = None


def _build(reps=1, single=False, fake_ag=False, g3=None, agm=None,
           probe=None):
    g3 = g3 or G3
    agm = agm or AGM
    probe = PROBE if probe is None else probe
    nc = bacc.Bacc("TRN2", target_bir_lowering=False, debug=False,
                   num_devices=1 if single else N_CORES)
    f32 = mybir.dt.float32
    f16 = mybir.dt.float16
    bf16 = mybir.dt.bfloat16
    fp8 = mybir.dt.float8e4
    AF = mybir.ActivationFunctionType
    DR = mybir.MatmulPerfMode.DoubleRow

    concat8 = nc.dram_tensor("concat8", [D, D], fp8, kind="ExternalInput").ap()
    concat8l = (nc.dram_tensor("concat8l", [D, D], fp8,
                               kind="ExternalInput").ap()
                if g3.startswith("res8") else None)
    g3w = {"f16": f16, "b16": bf16}.get(g3)
    concat16 = (nc.dram_tensor("concat16", [D, D], g3w,
                               kind="ExternalInput").ap()
                if g3w else None)
    wt = [nc.dram_tensor(f"w{g}t", [D, R], g3w if (g == 3 and g3w)
                         else fp8, kind="ExternalInput").ap()
          for g in range(1, 6)]
    w3lt = (nc.dram_tensor("w3lt", [D, R], fp8, kind="ExternalInput").ap()
            if g3.startswith("res8") else None)
    # b1..b4 (all *1024) concatenated along columns, bf16
    ba = nc.dram_tensor("ba", [R, 4 * D], bf16, kind="ExternalInput").ap()
    b5 = nc.dram_tensor("b5", [R, D], bf16, kind="ExternalInput").ap()
    cprev = nc.dram_tensor("cprev", [R, D], bf16, kind="ExternalInput").ap()

    ct_o = nc.dram_tensor("ct_o", [R, D], f16, kind="ExternalOutput").ap()
    ht_o = nc.dram_tensor("ht_o", [R, D], f16, kind="ExternalOutput").ap()
    yt_o = nc.dram_tensor("yt_o", [R, D], f16, kind="ExternalOutput").ap()

    rg = [list(range(N_CORES))]

    def mm_pairs(ps, w_sb, x_sb, m, start, stop):
        """K=2048 fp8 DoubleRow pass of one (weights, x) product."""
        for kk in range(PK // 2):
            lhs = (w_sb[:, 2 * kk * R:(2 * kk + 2) * R]
                   .rearrange("p (two m) -> p two m", two=2)
                   [:, :, m * 128:(m + 1) * 128])
            rhs = (x_sb[:, 2 * kk * NCOL:(2 * kk + 2) * NCOL]
                   .rearrange("p (two n) -> p two n", two=2))
            nc.tensor.matmul(ps[:], lhs, rhs,
                             start=start and kk == 0,
                             stop=stop and kk == PK // 2 - 1, perf_mode=DR)

    def mm_f16(ps, w_sb, x_sb, m):
        for k in range(PK):
            nc.tensor.matmul(
                ps[:],
                w_sb[:, k * R + m * 128:k * R + (m + 1) * 128],
                x_sb[:, k * NCOL:(k + 1) * NCOL],
                start=(k == 0), stop=(k == PK - 1))

    with tile.TileContext(nc) as tc:
        with (
            tc.tile_pool(name="wpool", bufs=1) as wpool,
            tc.tile_pool(name="xpool", bufs=2) as xpool,
            tc.tile_pool(name="hpool", bufs=2) as hpool,
            tc.tile_pool(name="bpool", bufs=1) as bpool,
            tc.tile_pool(name="opool", bufs=1) as opool,
            tc.tile_pool(name="gpool", bufs=1) as gpool,
            tc.tile_pool(name="zpool", bufs=1) as zpool,
            tc.tile_pool(name="spool", bufs=4) as spool,
            tc.tile_pool(name="psum", bufs=1, space="PSUM") as pp,
            tc.tile_pool(name="dram", bufs=1, space="DRAM") as dram,
        ):
            scr = gpool.tile([128, NCOL], bf16, name="scr", tag="scr",
                             bufs=2)

            def dup_v(a, b):
                if probe == "dve2":
                    nc.vector.tensor_add(scr[:], a, b)

            def dup_a(x, fn):
                if probe == "act2":
                    nc.scalar.activation(scr[:], x, fn)

            for rep in range(reps):
                w_sb = [wpool.tile([128, PK * R],
                                   g3w if (g == 2 and g3w) else fp8,
                                   name=f"w{g}sb", tag=f"w{g}sb")
                        for g in range(5)]
                w3l_sb = (wpool.tile([128, PK * R], fp8, name="w3lsb",
                                     tag="w3lsb")
                          if g3.startswith("res8") else None)
                w_loaded = [False] * 5

                def load_w(g):
                    if not w_loaded[g]:
                        w_loaded[g] = True
                        if g == 0:
                            # 4-k pieces so the PE can start on k=0 early
                            for k4 in range(0, PK, 4):
                                nc.scalar.dma_start(
                                    w_sb[g][:, k4 * R:(k4 + 4) * R]
                                    .rearrange("p (k m) -> p k m", m=R),
                                    wt[g][k4 * 128:(k4 + 4) * 128, :]
                                    .rearrange("(k p) m -> p k m", p=128))
                        else:
                            nc.scalar.dma_start(
                                w_sb[g][:].rearrange("p (k m) -> p k m", m=R),
                                wt[g].rearrange("(k p) m -> p k m", p=128))
                            if g == 2 and g3.startswith("res8"):
                                nc.scalar.dma_start(
                                    w3l_sb[:].rearrange("p (k m) -> p k m",
                                                        m=R),
                                    w3lt.rearrange("(k p) m -> p k m", p=128))

                # weight prefetch in first-use order (all on the scalar queue,
                # which carries only dependency-free constant loads)
                for g in (0, 1, 3, 2):
                    load_w(g)

                # phase-A constants: one DMA per 128-row half each
                ba_sb = [bpool.tile([128, 4 * D], bf16, name=f"ba{m}",
                                    tag=f"ba{m}") for m in range(NM)]
                cp_sb = [bpool.tile([128, D], bf16, name=f"cp{m}",
                                    tag=f"cp{m}") for m in range(NM)]
                b5_sb = [bpool.tile([128, D], bf16, name=f"b5_{m}",
                                    tag=f"b5_{m}") for m in range(NM)]
                for m in range(NM):
                    rsl = slice(m * 128, (m + 1) * 128)
                    nc.scalar.dma_start(ba_sb[m][:], ba[rsl, :])
                    nc.scalar.dma_start(cp_sb[m][:], cprev[rsl, :])
                # full-width output accumulation tiles (one store per half)
                ct_f = [opool.tile([128, D], f16, name=f"ctf{m}", tag=f"ctf{m}")
                        for m in range(NM)]
                ht_f = [opool.tile([128, D], f16, name=f"htf{m}", tag=f"htf{m}")
                        for m in range(NM)]

                ag_outs = []
                asp1 = "Local" if (single or fake_ag) else "Shared"
                if agm == "one":
                    ag1_in = dram.tile([R, D], fp8, name="ag1in", tag="ag1in")
                    ag1_out = dram.tile([D, D], fp8, name="ag1out",
                                        tag="ag1out", addr_space=asp1)
                # --- phase A: gates, ct, ht; all-gather ht by column chunk ---
                for n in range(NN):
                    csl = slice(n * NCOL, (n + 1) * NCOL)
                    x8_sb = xpool.tile([128, PK * NCOL], fp8, name=f"x8{n}",
                                       tag="x8sb")
                    if g3.startswith("res8"):
                        x2_sb = xpool.tile([128, PK * NCOL], fp8,
                                           name=f"x8l{n}", tag="x2sb")
                        x2_src = concat8l
                    elif g3w:
                        x2_sb = xpool.tile([128, PK * NCOL], g3w,
                                           name=f"x16{n}", tag="x2sb")
                        x2_src = concat16
                    else:
                        x2_sb = x2_src = None
                    if n == 0:
                        # 4-k pieces so gate0 starts early
                        for k4 in range(0, PK, 4):
                            nc.sync.dma_start(
                                x8_sb[:, k4 * NCOL:(k4 + 4) * NCOL]
                                .rearrange("p (k c) -> p k c", c=NCOL),
                                concat8[k4 * 128:(k4 + 4) * 128, csl]
                                .rearrange("(k p) c -> p k c", p=128))
                        for k8 in (range(0, PK, 8) if x2_sb is not None
                                   else ()):
                            nc.sync.dma_start(
                                x2_sb[:, k8 * NCOL:(k8 + 8) * NCOL]
                                .rearrange("p (k c) -> p k c", c=NCOL),
                                x2_src[k8 * 128:(k8 + 8) * 128, csl]
                                .rearrange("(k p) c -> p k c", p=128))
                    else:
                        nc.sync.dma_start(
                            x8_sb[:].rearrange("p (k c) -> p k c", c=NCOL),
                            concat8[:, csl].rearrange("(k p) c -> p k c", p=128))
                        if x2_sb is not None:
                            nc.sync.dma_start(
                                x2_sb[:].rearrange("p (k c) -> p k c", c=NCOL),
                                x2_src[:, csl].rearrange("(k p) c -> p k c",
                                                         p=128))

                    asp = asp1
                    if agm == "one":
                        ag_in = None
                        ag_outs.append([(ag1_out, n, NCOL)])
                    elif n == NN - 1 and not (single or fake_ag):
                        h2 = NCOL // 2
                        ag_in = [dram.tile([R, h2], fp8, name=f"agin{n}{s}",
                                           tag=f"agin{n}{s}")
                                 for s in ("a", "b")]
                        ag_out = [dram.tile([D, h2], fp8, name=f"agout{n}{s}",
                                            tag=f"agout{n}{s}", addr_space=asp)
                                  for s in ("a", "b")]
                        ag_outs.append([(ag_out[0], 0, h2),
                                        (ag_out[1], h2, h2)])
                    else:
                        ag_in = dram.tile([R, NCOL], fp8, name=f"agin{n}",
                                          tag=f"agin{n}")
                        ag_out = dram.tile([D, NCOL], fp8, name=f"agout{n}",
                                           tag=f"agout{n}", addr_space=asp)
                        ag_outs.append([(ag_out, 0, NCOL)])

                    # fp8 gates first; the (slower) fp16 tanh gate last, with
                    # the independent elementwise work interleaved behind it
                    FNS = [AF.Sigmoid, AF.Sigmoid, AF.Tanh, AF.Sigmoid]
                    for m in range(NM):
                        rsl = slice(m * 128, (m + 1) * 128)
                        ps = [None] * 4
                        for g in (0, 1, 3, 2):
                            p = pp.tile([128, NCOL], f32, name=f"ps{g}",
                                        tag=f"ps{g}")
                            if g == 2 and g3 == "res8":
                                mm_pairs(p, w_sb[2], x8_sb, m, True, False)
                                mm_pairs(p, w_sb[2], x2_sb, m, False, False)
                                mm_pairs(p, w3l_sb, x8_sb, m, False, True)
                            elif g == 2 and g3 == "res8b":
                                mm_pairs(p, w_sb[2], x8_sb, m, True, True)
                                p2 = pp.tile([128, NCOL], f32, name="ps2b",
                                             tag="ps2b")
                                mm_pairs(p2, w_sb[2], x2_sb, m, True, False)
                                mm_pairs(p2, w3l_sb, x8_sb, m, False, True)
                                ps2b = p2
                            elif g == 2 and g3w:
                                mm_f16(p, w_sb[2], x2_sb, m)
                            else:
                                mm_pairs(p, w_sb[g], x8_sb, m, True, True)
                            ps[g] = p
                        load_w(4)  # w5 queued early; needed only in phase C

                        acts = [None] * 4

                        def bias_act(g):
                            psg = ps[g][:]
                            if g == 2 and g3 == "res8b":
                                t = gpool.tile([128, NCOL], f32, name="g3t",
                                               tag="g3t")
                                nc.vector.tensor_add(t[:], psg, ps2b[:])
                                psg = t[:]
                            pre = gpool.tile([128, NCOL], bf16, name=f"pre{g}",
                                             tag=f"pre{g}")
                            nc.vector.tensor_add(
                                pre[:], psg,
                                ba_sb[m][:, g * D + n * NCOL:
                                         g * D + (n + 1) * NCOL])
                            dup_v(psg, ba_sb[m][:, g * D + n * NCOL:
                                                g * D + (n + 1) * NCOL])
                            act = gpool.tile([128, NCOL], bf16, name=f"act{g}",
                                             tag=f"act{g}")
                            sc = 1.0 if (g == 2 and g3w) else 1.0 / SS
                            nc.scalar.activation(act[:], pre[:], FNS[g],
                                                 scale=sc)
                            dup_a(pre[:], FNS[g])
                            acts[g] = act

                        for g in (0, 1, 3):
                            bias_act(g)
                        # t1 only needs gate0 — runs while the fp16 gate mms
                        t1 = gpool.tile([128, NCOL], bf16, name="t1", tag="t1")
                        nc.vector.tensor_mul(t1[:], acts[0][:],
                                             cp_sb[m][:, csl])
                        dup_v(acts[0][:], cp_sb[m][:, csl])
                        bias_act(2)
                        t2 = gpool.tile([128, NCOL], bf16, name="t2", tag="t2")
                        nc.vector.tensor_mul(t2[:], acts[1][:], acts[2][:])
                        dup_v(acts[1][:], acts[2][:])
                        nc.vector.tensor_add(ct_f[m][:, csl], t1[:], t2[:])
                        dup_v(t1[:], t2[:])

                        th = gpool.tile([128, NCOL], bf16, name="th", tag="th")
                        nc.scalar.activation(th[:], ct_f[m][:, csl], AF.Tanh)
                        dup_a(ct_f[m][:, csl], AF.Tanh)
                        nc.vector.tensor_mul(ht_f[m][:, csl], acts[3][:], th[:])
                        dup_v(acts[3][:], th[:])
                        htb = gpool.tile([128, NCOL], fp8, name="htb",
                                         tag="htb", bufs=2)
                        nc.scalar.activation(htb[:], ht_f[m][:, csl], AF.Copy,
                                             scale=S)
                        if agm == "one":
                            nc.gpsimd.dma_start(ag1_in[rsl, csl], htb[:])
                        elif isinstance(ag_in, list):
                            h2 = NCOL // 2
                            nc.gpsimd.dma_start(ag_in[0][rsl, :], htb[:, :h2])
                            nc.gpsimd.dma_start(ag_in[1][rsl, :], htb[:, h2:])
                        else:
                            nc.gpsimd.dma_start(ag_in[rsl, :], htb[:])
                        if n == NN - 1:
                            nc.gpsimd.dma_start(ct_o[rsl, :], ct_f[m][:])
                            nc.gpsimd.dma_start(ht_o[rsl, :], ht_f[m][:])

                    if agm == "one":
                        if n == NN - 1:
                            if single or fake_ag:
                                for blk in range(N_CORES):
                                    eng = (nc.sync if blk % 2 == 0
                                           else nc.scalar)
                                    eng.dma_start(
                                        ag1_out[blk * R:(blk + 1) * R, :],
                                        ag1_in[:])
                            else:
                                nc.gpsimd.collective_compute(
                                    "AllGather", mybir.AluOpType.bypass,
                                    replica_groups=rg,
                                    ins=[ag1_in.opt()], outs=[ag1_out.opt()])
                    elif single or fake_ag:
                        # stand-in for the AllGather: equivalent local HBM
                        # write volume so TimelineSim sees the same DMA load
                        for blk in range(N_CORES):
                            eng = nc.sync if blk % 2 == 0 else nc.scalar
                            eng.dma_start(
                                ag_out[blk * R:(blk + 1) * R, :], ag_in[:])
                    elif isinstance(ag_in, list):
                        for agi, ago in zip(ag_in, ag_out):
                            nc.gpsimd.collective_compute(
                                "AllGather", mybir.AluOpType.bypass,
                                replica_groups=rg,
                                ins=[agi.opt()], outs=[ago.opt()])
                    else:
                        nc.gpsimd.collective_compute(
                            "AllGather", mybir.AluOpType.bypass,
                            replica_groups=rg,
                            ins=[ag_in.opt()], outs=[ag_out.opt()])

                # --- phase C: z5 = w5 @ ht + b5, then row softmax ---
                for m in range(NM):
                    nc.scalar.dma_start(b5_sb[m][:],
                                        b5[m * 128:(m + 1) * 128, :])
                exs = [zpool.tile([128, D], f32, name=f"ex{m}", tag=f"ex{m}")
                       for m in range(NM)]
                sms = [[spool.tile([128, 1], f32, name=f"sm{m}_{n}",
                                   tag=f"sm{m}_{n}") for n in range(NN)]
                       for m in range(NM)]
                for n in range(NN):
                    csl = slice(n * NCOL, (n + 1) * NCOL)
                    h_sb = hpool.tile([128, PK * NCOL], fp8, name=f"h{n}",
                                      tag="hsb")
                    for ago, coff, cw in ag_outs[n]:
                        if agm == "one":
                            nc.scalar.dma_start(
                                h_sb[:].rearrange("p (k c) -> p k c", c=NCOL),
                                ago[:, csl].rearrange("(k p) c -> p k c",
                                                      p=128))
                        else:
                            nc.scalar.dma_start(
                                h_sb[:].rearrange("p (k c) -> p k c", c=NCOL)
                                [:, :, coff:coff + cw],
                                ago.rearrange("(k p) c -> p k c", p=128))
                    for m in range(NM):
                        p5 = pp.tile([128, NCOL], f32, name="ps5", tag="ps5",
                                     bufs=2)
                        mm_pairs(p5, w_sb[4], h_sb, m, True, True)
                        z5 = gpool.tile([128, NCOL], f32, name="z5", tag="z5",
                                        bufs=2)
                        nc.vector.tensor_add(z5[:], p5[:],
                                             b5_sb[m][:, csl])
                        # chunked exp (scale folds away the 1024x) with
                        # per-chunk row-sum; logits are bounded (|z| < ~1)
                        # so exp without max subtraction is safe
                        nc.scalar.activation(exs[m][:, csl], z5[:],
                                             AF.Exp, scale=1.0 / SS,
                                             accum_out=sms[m][n][:])

                yt_f = [opool.tile([128, D], f16, name=f"ytf{m}", tag=f"ytf{m}")
                        for m in range(NM)]
                for m in range(NM):
                    s01 = spool.tile([128, 1], f32, name="s01", tag="s01")
                    nc.vector.tensor_add(s01[:], sms[m][0][:], sms[m][1][:])
                    s23 = spool.tile([128, 1], f32, name="s23", tag="s23")
                    nc.vector.tensor_add(s23[:], sms[m][2][:], sms[m][3][:])
                    sm_t = spool.tile([128, 1], f32, name="sm_t", tag="sm_t")
                    nc.vector.tensor_add(sm_t[:], s01[:], s23[:])
                    rs = spool.tile([128, 1], f32, name="rs", tag="rs")
                    nc.vector.reciprocal(rs[:], sm_t[:])
                    for j in range(NN):
                        jsl = slice(j * NCOL, (j + 1) * NCOL)
                        nc.vector.tensor_scalar_mul(yt_f[m][:, jsl],
                                                    exs[m][:, jsl], rs[:])
                    nc.gpsimd.dma_start(yt_o[m * 128:(m + 1) * 128, :],
                                        yt_f[m][:])

    nc.compile()
    return nc


_RUNNER = None


def _build_runner(nc):
    """Cached jit-compiled SPMD executor mirroring run_bass_kernel_spmd's
    axon/PJRT path, so repeat kernel() calls skip retracing."""
    import jax
    from jax.sharding import Mesh, PartitionSpec, NamedSharding
    from jax.experimental.shard_map import shard_map
    from concourse.bass2jax import (_bass_exec_p, install_neuronx_cc_hook,
                                    partition_id_tensor)

    install_neuronx_cc_hook()
    partition_name = (nc.partition_id_tensor.name
                      if nc.partition_id_tensor else None)
    in_names, out_names, out_avals = [], [], []
    for alloc in nc.m.functions[0].allocations:
        if not isinstance(alloc, mybir.MemoryLocationSet):
            continue
        name = alloc.memorylocations[0].name
        if alloc.kind == "ExternalInput":
            if name != partition_name:
                in_names.append(name)
        elif alloc.kind == "ExternalOutput":
            out_names.append(name)
            out_avals.append(jax.core.ShapedArray(
                tuple(alloc.tensor_shape), mybir.dt.np(alloc.dtype)))
    n_params, n_outs = len(in_names), len(out_names)
    all_in = tuple(in_names + out_names
                   + ([partition_name] if partition_name else []))

    def _body(*args):
        operands = list(args)
        if partition_name is not None:
            operands.append(partition_id_tensor())
        return tuple(_bass_exec_p.bind(
            *operands, out_avals=tuple(out_avals), in_names=all_in,
            out_names=tuple(out_names), lowering_input_output_aliases=(),
            sim_require_finite=True, sim_require_nnan=True, nc=nc))

    devices = jax.devices()[:N_CORES]
    mesh = Mesh(np.asarray(devices), ("core",))
    specs = (PartitionSpec("core"),) * (n_params + n_outs)
    fn = jax.jit(
        shard_map(_body, mesh=mesh, in_specs=specs,
                  out_specs=(PartitionSpec("core"),) * n_outs,
                  check_rep=False),
        donate_argnums=tuple(range(n_params, n_params + n_outs)),
        keep_unused=True)
    sh = NamedSharding(mesh, PartitionSpec("core"))
    zeros = [np.zeros((N_CORES * av.shape[0], *av.shape[1:]), av.dtype)
             for av in out_avals]

    def run(in_maps):
        gin = [jax.device_put(
            np.concatenate([in_maps[c][nm] for c in range(N_CORES)], 0), sh)
            for nm in in_names]
        gz = [jax.device_put(z, sh) for z in zeros]
        out = fn(*gin, *gz)
        got = {nm: np.asarray(o) for nm, o in zip(out_names, out)}
        return [{nm: got[nm].reshape(N_CORES, -1, got[nm].shape[-1])[c]
                 for nm in out_names} for c in range(N_CORES)]

    return run


def _make_in_maps(inputs, g3=None):
    g3 = g3 or G3
    inp = {k: np.asarray(v) for k, v in inputs.items()}
    concat = np.concatenate([inp["hPrev"], inp["xt"]], axis=0).astype(np.float32)
    concat8 = (concat * S).astype(F8)
    in_maps = []
    for i in range(N_CORES):
        r = slice(i * R, (i + 1) * R)
        m = {"concat8": concat8,
             "cprev": np.ascontiguousarray(inp["cPrev"][r]).astype(BF16)}
        if g3.startswith("res8"):
            m["concat8l"] = (concat * S
                             - concat8.astype(np.float32)).astype(F8)
        elif g3 in ("f16", "b16"):
            m["concat16"] = concat.astype(
                np.float16 if g3 == "f16" else BF16)
        bs = []
        for g in range(1, 6):
            wT = np.ascontiguousarray(inp[f"w{g}"][r].astype(np.float32).T)
            bg = np.ascontiguousarray(inp[f"b{g}"][r]).astype(np.float32)
            ws = wT * S
            if g == 3 and g3 in ("f16", "b16"):
                m["w3t"] = wT.astype(
                    np.float16 if g3 == "f16" else BF16)
                bs.append(bg)          # unscaled: 16-bit gate psum is unscaled
                continue
            m[f"w{g}t"] = ws.astype(F8)
            if g == 3 and g3.startswith("res8"):
                m["w3lt"] = (ws - m["w3t"].astype(np.float32)).astype(F8)
            if g == 5:
                m["b5"] = (bg * SS).astype(BF16)
            else:
                bs.append(bg * SS)
        m["ba"] = np.concatenate(bs, axis=1).astype(BF16)
        in_maps.append(m)
    return in_maps


def kernel(**inputs):
    global _CACHE, _RUNNER
    if _CACHE is None:
        _CACHE = _build()
    nc = _CACHE
    in_maps = _make_in_maps(inputs)

    results = None
    if _RUNNER is not False:
        try:
            if _RUNNER is None:
                _RUNNER = _build_runner(nc)
            results = _RUNNER(in_maps)
        except Exception:
            _RUNNER = False  # fall back permanently for this process
    if results is None:
        res = bass_utils.run_bass_kernel_spmd(nc, in_maps,
                                              core_ids=list(range(N_CORES)))
        results = res.results

    ct = np.concatenate([results[i]["ct_o"] for i in range(N_CORES)], 0)
    ht = np.concatenate([results[i]["ht_o"] for i in range(N_CORES)], 0)
    yt = np.concatenate([results[i]["yt_o"] for i in range(N_CORES)], 0)
    return (ct.astype(np.float32), ht.astype(np.float32),
            yt.astype(np.float32))
